# revision 17
# baseline (speedup 1.0000x reference)
"""SAN Bottleneck (pairwise self-attention) Trainium2 kernel.

Sharding: 8 cores = 2 batches x 4 row-blocks of 14 rows (H=56). Each core
receives a reflect-padded input slice (20 rows x 62 cols), so the 7x7
unfold needs no runtime halo exchange and no edge special-casing.

Per-core pipeline (all batchnorms folded into per-channel scale/bias on host):
  bn1+relu -> x1/x2/x3 1x1 convs (matmuls)
  feat = relu(x1 - shifted-window(x2))            (fp16, DVE/GPSIMD)
  mm1 66->64 (fp16), PSUM packed 2 column-halves onto 128 partitions so
  the relu+bias eviction (ACT) covers half the free size
  mm2 64->128 with 4x-replicated head weights -> exp+bias (ACT)
  softmax normalizer + aggregation: shifted-window products and pairwise
  tree adds over the 49 taps, with 4-free-dim APs so each 7x{4,3}-tap
  group is a single DVE/GPSIMD instruction
  bn2+relu -> wc conv + bias + identity residual.

The x3/aggregation channels are permuted host-side (s-split: tile t,
partition p <-> channel 8*(p//4)+4t+(p%4)) so one 4x-replicated exp tensor
serves both 128-channel tiles without any partition broadcast. The position
branch (batch independent) is precomputed on host as relu(bn(subp)) and DMA'd
into feat rows 64:65. Input x ships as bf16 (residual tolerance allows it).
"""

import numpy as np
import ml_dtypes

bf16_np = ml_dtypes.bfloat16

K = 7
PAD = 3
EPS = 1e-5
B, C, H, W = 2, 256, 56, 56
RB = 14              # rows per core
NQ = RB * W          # 784
ROWS = RB + 2 * PAD  # 20
WP = W + 2 * PAD     # 62
K2 = K * K
CHUNKS = [(0, 4), (4, 4), (8, 3), (11, 3)]

_BUILD_CACHE = {}


def _perm_channels():
    perm = np.zeros(256, np.int64)
    for t in range(2):
        for p in range(128):
            perm[t * 128 + p] = 8 * (p // 4) + 4 * t + (p % 4)
    return perm


def _build_program():
    if "nc" in _BUILD_CACHE:
        return _BUILD_CACHE["nc"]
    import concourse.bass as bass
    import concourse.bacc as bacc
    import concourse.tile as tile
    import concourse.mybir as mybir
    from contextlib import ExitStack

    f32 = mybir.dt.float32
    f16 = mybir.dt.float16
    bf16 = mybir.dt.bfloat16
    Alu = mybir.AluOpType
    Act = mybir.ActivationFunctionType

    nc = bacc.Bacc("TRN2", target_bir_lowering=False, num_devices=8)

    xp_d = nc.dram_tensor("xp", [2, 128, ROWS, WP], bf16, kind="ExternalInput")
    rsubp_d = nc.dram_tensor("rsubp", [2, K2, NQ], f16, kind="ExternalInput")
    w1T_d = nc.dram_tensor("w1T", [2, 128, 64], bf16, kind="ExternalInput")
    w2T_d = nc.dram_tensor("w2T", [2, 128, 64], bf16, kind="ExternalInput")
    w3T_d = nc.dram_tensor("w3T", [2, 128, 2, 128], bf16, kind="ExternalInput")
    wcT_d = nc.dram_tensor("wcT", [2, 128, 2, 128], bf16, kind="ExternalInput")
    cw1T_d = nc.dram_tensor("cw1T", [66, 64], f16, kind="ExternalInput")
    cw2T_d = nc.dram_tensor("cw2T", [128, 128], f16, kind="ExternalInput")
    scal_d = nc.dram_tensor("scal", [128, 14], f32, kind="ExternalInput")
    y_d = nc.dram_tensor("y", [2, 128, RB, W], f32, kind="ExternalOutput")

    def win_ap(base, elem_off, ndj, nr):
        # [P][ndj dj (stride 2)][nr rows (stride WP)][56 cols] into a flat
        # padded [P, ROWS*WP] tile
        return bass.AP(
            tensor=base.tensor,
            offset=base.offset + elem_off,
            ap=[base.ap[0], [2, ndj], [WP, nr], [1, W]],
        )

    def kq_ap(base3, k0, ndj, nqc, nr):
        # [P][ndj (stride 2*nqc)][nr][56] into a [P, 49, nqc] tile at tap k0
        return bass.AP(
            tensor=base3.tensor,
            offset=base3.offset + k0 * nqc,
            ap=[base3.ap[0], [2 * nqc, ndj], [W, nr], [1, W]],
        )

    with tile.TileContext(nc) as tc, ExitStack() as stack:
        consts = stack.enter_context(tc.tile_pool(name="consts", bufs=1))
        xpp = stack.enter_context(tc.tile_pool(name="xpp", bufs=1))
        headsb = stack.enter_context(tc.tile_pool(name="headsb", bufs=1))

        w1s = consts.tile([128, 2, 64], bf16, tag="w1s")
        w2s = consts.tile([128, 2, 64], bf16, tag="w2s")
        w3s = consts.tile([128, 2, 2, 128], bf16, tag="w3s")
        wcs = consts.tile([128, 2, 2, 128], bf16, tag="wcs")
        cw1s = consts.tile([66, 64], f16, tag="cw1s")
        cw2s = consts.tile([128, 128], f16, tag="cw2s")
        scals = consts.tile([128, 14], f32, tag="scals")
        for kt in range(2):
            nc.sync.dma_start(out=w1s[:, kt, :], in_=w1T_d[kt])
            nc.sync.dma_start(out=w2s[:, kt, :], in_=w2T_d[kt])
            nc.sync.dma_start(out=w3s[:, kt, :, :], in_=w3T_d[kt])
            nc.sync.dma_start(out=wcs[:, kt, :, :], in_=wcT_d[kt])
        nc.sync.dma_start(out=cw1s[:], in_=cw1T_d[:])
        nc.sync.dma_start(out=cw2s[:], in_=cw2T_d[:])
        nc.sync.dma_start(out=scals[:], in_=scal_d[:])

        a1 = [scals[:, 0:1], scals[:, 1:2]]
        b1f = [scals[:, 2:3], scals[:, 3:4]]
        b1p = scals[0:64, 4:5]
        b2p = scals[0:64, 5:6]
        b2f = scals[:, 6:7]          # replicated to 128 rows host-side
        cb2r = scals[:, 7:8]
        a3p = [scals[:, 8:9], scals[:, 9:10]]
        b3fp = [scals[:, 10:11], scals[:, 11:12]]
        bcb = [scals[:, 12:13], scals[:, 13:14]]

        xps = [xpp.tile([128, ROWS, WP], bf16, tag=f"xp{t}", name=f"xp{t}") for t in range(2)]
        for t in range(2):
            nc.sync.dma_start(out=xps[t][:], in_=xp_d[t])
        obn = [headsb.tile([128, ROWS * WP], bf16, tag=f"obn{t}", name=f"obn{t}") for t in range(2)]
        for t in range(2):
            nc.scalar.activation(
                out=obn[t][:],
                in_=xps[t][:].rearrange("p r w -> p (r w)"),
                func=Act.Relu, bias=b1f[t], scale=a1[t])

        x1s = headsb.tile([64, RB, W], f16, tag="x1s")
        x2p = headsb.tile([64, ROWS * WP], f16, tag="x2p")
        x3ps = headsb.tile([128, 2, ROWS * WP], f16, tag="x3ps")

        with tc.tile_pool(name="pshead", bufs=4, space="PSUM") as pshead:
            ccuts = [(0, 416), (416, 416), (832, 408)]
            for (o0, n) in ccuts:
                ps = pshead.tile([64, 416], f32, tag="ps64")
                for kt in range(2):
                    nc.tensor.matmul(
                        ps[:, :n], w2s[:, kt, :],
                        obn[kt][:, o0:o0 + n],
                        start=(kt == 0), stop=(kt == 1))
                nc.scalar.activation(out=x2p[:, o0:o0 + n], in_=ps[:, :n],
                                     func=Act.Identity, bias=b2p, scale=1.0)
            for half in range(2):
                ps = pshead.tile([64, 416], f32, tag="ps64")
                for kt in range(2):
                    rhs = obn[kt][:].rearrange("p (r w) -> p r w", w=WP)[
                        :, 3 + 7 * half:3 + 7 * (half + 1), 3:3 + W]
                    nc.tensor.matmul(ps[:, :392], w1s[:, kt, :],
                                     rhs,
                                     start=(kt == 0), stop=(kt == 1))
                nc.scalar.activation(
                    out=x1s[:, 7 * half:7 * (half + 1), :],
                    in_=ps[:, :392].rearrange("p (r w) -> p r w", w=W),
                    func=Act.Identity, bias=b1p, scale=1.0)
            for ot in range(2):
                for (o0, n) in ccuts:
                    ps = pshead.tile([128, 416], f32, tag="ps128")
                    for kt in range(2):
                        nc.tensor.matmul(
                            ps[:, :n], w3s[:, kt, ot, :],
                            obn[kt][:, o0:o0 + n],
                            start=(kt == 0), stop=(kt == 1))
                    nc.scalar.activation(out=x3ps[:, ot, o0:o0 + n],
                                         in_=ps[:, :n], func=Act.Copy)

        featp = stack.enter_context(tc.tile_pool(name="featp", bufs=2))
        h2p = stack.enter_context(tc.tile_pool(name="h2p", bufs=1))
        e4p = stack.enter_context(tc.tile_pool(name="e4p", bufs=2))
        prodp = stack.enter_context(tc.tile_pool(name="prodp", bufs=1))
        smallp = stack.enter_context(tc.tile_pool(name="smallp", bufs=2))
        zscp = stack.enter_context(tc.tile_pool(name="zscp", bufs=1))
        ps1p = stack.enter_context(tc.tile_pool(name="ps1p", bufs=2, space="PSUM"))
        ps2p = stack.enter_context(tc.tile_pool(name="ps2p", bufs=2, space="PSUM"))

        chunk_state = {}

        def ksum_tree(eng, t):
            for (a, b, n) in [(0, 24, 24), (0, 12, 12), (0, 6, 6), (0, 3, 3)]:
                eng.tensor_tensor(out=t[:, a:a + n, :], in0=t[:, a:a + n, :],
                                  in1=t[:, b:b + n, :], op=Alu.add)
            for b in (1, 2, 48):
                eng.tensor_tensor(out=t[:, 0, :], in0=t[:, 0, :],
                                  in1=t[:, b, :], op=Alu.add)

        def h2_block(nb2, tail_base, m):
            # map global 512-col block m of mm1's output onto (psum-half
            # partition start, h2 column start) under the pack-2 layout
            t, r = divmod(m, 4)
            if t < nb2:
                return 64 * (r // 2), t * 1024 + 512 * (r % 2)
            mt = m - 4 * nb2
            return 0, tail_base + 512 * mt

        def phase1(ci):
            (r0c, nr) = CHUNKS[ci]
            nqc = nr * W
            vc = K2 * nqc
            nb2 = vc // 2048
            tail_base = nb2 * 1024
            tail = vc - nb2 * 2048
            feat = featp.tile([66, K2, nqc], f16, tag="feat", name=f"feat{ci}")
            nc.sync.dma_start(out=feat[64:66, :, :],
                                in_=rsubp_d[:, :, r0c * W:r0c * W + nqc])

            fv = feat[0:64]
            x1v = x1s[:, r0c:r0c + nr, :]
            for di in range(K):
                for par in range(2):
                    ndj = 4 if par == 0 else 3
                    x2w = win_ap(x2p[:], (r0c + di) * WP + par, ndj, nr)
                    x1w = bass.AP(tensor=x1v.tensor, offset=x1v.offset,
                                  ap=[x1v.ap[0], [0, ndj], x1v.ap[1], x1v.ap[2]])
                    outw = kq_ap(fv, di * K + par, ndj, nqc, nr)
                    eng = nc.vector if par == 0 else nc.gpsimd
                    eng.tensor_tensor(out=outw, in0=x1w, in1=x2w,
                                      op=Alu.subtract)
            for bq in range(4):
                ks = (K2 * bq) // 4, (K2 * (bq + 1)) // 4
                nc.vector.tensor_scalar_max(
                    out=feat[0:64, ks[0]:ks[1], :].rearrange("p a b -> p (a b)"),
                    in0=feat[0:64, ks[0]:ks[1], :].rearrange("p a b -> p (a b)"),
                    scalar1=0.0)

            featf = feat[:].rearrange("p a b -> p (a b)")
            h2 = h2p.tile([128, tail_base + max(tail, 0)], f16, tag="h2")
            for t in range(nb2):
                ps1 = ps1p.tile([128, 1024], f32, tag="ps1")
                for r in range(4):
                    j0 = t * 2048 + r * 512
                    nc.tensor.matmul(
                        ps1[64 * (r // 2):64 * (r // 2) + 64,
                            512 * (r % 2):512 * (r % 2) + 512],
                        cw1s[:], featf[:, j0:j0 + 512],
                        start=True, stop=True)
                nc.scalar.activation(out=h2[:, t * 1024:(t + 1) * 1024],
                                     in_=ps1[:, :], func=Act.Relu,
                                     bias=b2f, scale=1.0)
            if tail:
                ps1 = ps1p.tile([128, 1024], f32, tag="ps1")
                for s in range(0, tail, 512):
                    sn = min(512, tail - s)
                    nc.tensor.matmul(ps1[0:64, s:s + sn], cw1s[:],
                                     featf[:, nb2 * 2048 + s:nb2 * 2048 + s + sn],
                                     start=True, stop=True)
                nc.scalar.activation(out=h2[0:64, tail_base:tail_base + tail],
                                     in_=ps1[0:64, :tail], func=Act.Relu,
                                     bias=b2f[0:64], scale=1.0)

            e4 = e4p.tile([128, K2, nqc], f16, tag="e4")
            e4f = e4[:].rearrange("p a b -> p (a b)")
            nblk = (vc + 511) // 512
            m = 0
            while m < nblk:
                take = min(2, nblk - m)
                ps2 = ps2p.tile([128, 1024], f32, tag="ps2")
                w = 0
                for i in range(take):
                    bi = m + i
                    n = min(512, vc - bi * 512)
                    php, hcol = h2_block(nb2, tail_base, bi)
                    nc.tensor.matmul(ps2[:, w:w + n], cw2s[php:php + 64, :],
                                     h2[php:php + 64, hcol:hcol + n],
                                     start=True, stop=True)
                    w += n
                nc.scalar.activation(out=e4f[:, m * 512:m * 512 + w],
                                     in_=ps2[:, :w],
                                     func=Act.Exp, bias=cb2r, scale=1.0)
                m += take

            chunk_state[ci] = (e4,)

        def phase2(ci):
            (r0c, nr) = CHUNKS[ci]
            nqc = nr * W
            (e4,) = chunk_state[ci]
            prods = []
            for ot in range(2):
                prodt = prodp.tile([128, K2, nqc], f16, tag=f"prod{ot}",
                                   name=f"prod{ot}")
                prods.append(prodt)
                for di in range(K):
                    for par in range(2):
                        ndj = 4 if par == 0 else 3
                        sv = x3ps[:, ot, :]
                        k0 = di * K + par
                        x3w = bass.AP(
                            tensor=sv.tensor,
                            offset=sv.offset + (r0c + di) * WP + par,
                            ap=[sv.ap[0], [2, ndj], [WP, nr], [1, W]])
                        e4w = kq_ap(e4[:], k0, ndj, nqc, nr)
                        outw = kq_ap(prods[ot][:], k0, ndj, nqc, nr)
                        nc.vector.tensor_tensor(out=outw, in0=e4w, in1=x3w,
                                                op=Alu.mult)
            ksum_tree(nc.vector, prods[0])
            ksum_tree(nc.vector, prods[1])
            zsc = zscp.tile([128, 24, nqc], f16, tag="zsc", name=f"zsc{ci}")
            nc.gpsimd.tensor_tensor(out=zsc[:, :, :], in0=e4[:, 0:24, :],
                                    in1=e4[:, 24:48, :], op=Alu.add)
            for (a, b, n) in [(0, 12, 12), (0, 6, 6), (0, 3, 3)]:
                nc.gpsimd.tensor_tensor(out=zsc[:, a:a + n, :],
                                        in0=zsc[:, a:a + n, :],
                                        in1=zsc[:, b:b + n, :], op=Alu.add)
            for bb in (1, 2):
                nc.gpsimd.tensor_tensor(out=zsc[:, 0, :], in0=zsc[:, 0, :],
                                        in1=zsc[:, bb, :], op=Alu.add)
            nc.gpsimd.tensor_tensor(out=zsc[:, 0, :], in0=zsc[:, 0, :],
                                    in1=e4[:, 48, :], op=Alu.add)

            zf = smallp.tile([128, nqc], f32, tag="zf")
            rz = smallp.tile([128, nqc], f32, tag="rz")
            nc.vector.tensor_copy(out=zf[:], in_=zsc[:, 0, :])
            nc.vector.reciprocal(out=rz[:], in_=zf[:])

            outb = []
            for ot in range(2):
                ob = smallp.tile([128, nqc], f32, tag=f"ob{ot}", name=f"ob{ot}")
                ob2 = smallp.tile([128, nqc], bf16, tag=f"ob2{ot}", name=f"ob2{ot}")
                outb.append(ob2)
                nc.vector.scalar_tensor_tensor(
                    out=ob[:], in0=prods[ot][:, 0, :], scalar=1.0, in1=rz[:],
                    op0=Alu.mult, op1=Alu.mult)
                nc.scalar.activation(out=ob2[:], in_=ob[:], func=Act.Relu,
                                     bias=b3fp[ot], scale=a3p[ot])

            for oo in range(2):
                psw = ps2p.tile([128, 1024], f32, tag="ps2")
                for kt in range(2):
                    nc.tensor.matmul(psw[:, :nqc], wcs[:, kt, oo, :],
                                     outb[kt][:],
                                     start=(kt == 0), stop=(kt == 1))
                ysb = smallp.tile([128, nqc], f32, tag=f"ysb{oo}", name=f"ysb{oo}")
                xi = xps[oo][:, PAD + r0c:PAD + r0c + nr, PAD:PAD + W]
                nc.vector.scalar_tensor_tensor(
                    out=ysb[:], in0=psw[:, :nqc], scalar=bcb[oo], in1=xi,
                    op0=Alu.add, op1=Alu.add)
                nc.sync.dma_start(
                    out=y_d[oo][:, r0c:r0c + nr, :],
                    in_=ysb[:].rearrange("p (r w) -> p r w", w=W))

        phase1(0)
        for ci in range(1, len(CHUNKS)):
            phase1(ci)
            phase2(ci - 1)
        phase2(len(CHUNKS) - 1)

    nc.compile()
    _BUILD_CACHE["nc"] = nc
    return nc


def _host_prep(inputs):
    f = {k: np.asarray(v, np.float32) for k, v in inputs.items()}

    def fold(n):
        a = f[n + "_g"] / np.sqrt(f[n + "_rv"] + EPS)
        return a, f[n + "_b"] - f[n + "_rm"] * a

    a1, b1f = fold("bn1")
    ac, bc1 = fold("cwbn1")
    a2, b2f = fold("cwbn2")
    a3, b3f = fold("bn2")

    W1p = ac[:64, None] * f["w1"]
    b1p = ac[:64] * f["b1"] + bc1[:64]
    W2p = ac[:64, None] * f["w2"]
    b2p = ac[:64] * f["b2"]
    cw1p = a2[:, None] * f["cw1"]

    perm = _perm_channels()
    w3p = f["w3"][perm]
    a3p = a3[perm]
    b3fp = b3f[perm]
    rep = np.arange(128) // 4
    cw2r = f["cw2"][rep]
    cb2r = f["cb2"][rep]

    locw = np.tile(np.linspace(-1.0, 1.0, W, dtype=np.float32)[None, :], (H, 1))
    loch = np.tile(np.linspace(-1.0, 1.0, H, dtype=np.float32)[:, None], (1, W))
    loc = np.stack([locw, loch], 0)
    p = np.einsum("chw,oc->ohw", loc, f["pw"]) + f["pb"][:, None, None]
    pp = np.pad(p, ((0, 0), (PAD, PAD), (PAD, PAD)), mode="reflect")
    pu = np.stack([pp[:, i:i + H, j:j + W] for i in range(K) for j in range(K)], 1)
    subp = p[:, None] - pu
    rsubp = np.maximum(ac[64:66, None, None, None] * subp
                       + bc1[64:66, None, None, None], 0).astype(np.float16)

    xpad = np.pad(f["x"], ((0, 0), (0, 0), (PAD, PAD), (PAD, PAD)), mode="reflect")

    w1T = np.ascontiguousarray(W1p.T).reshape(2, 128, 64).copy()
    w2T = np.ascontiguousarray(W2p.T).reshape(2, 128, 64).copy()
    w3T = np.empty((2, 128, 2, 128), np.float32)
    wcT = np.empty((2, 128, 2, 128), np.float32)
    wc_perm = f["wc"][:, perm]
    for kt in range(2):
        for ot in range(2):
            w3T[kt, :, ot, :] = w3p[ot * 128:(ot + 1) * 128,
                                    kt * 128:(kt + 1) * 128].T
            wcT[kt, :, ot, :] = wc_perm[ot * 128:(ot + 1) * 128,
                                        kt * 128:(kt + 1) * 128].T
    cw1T = np.ascontiguousarray(cw1p.T).astype(np.float16)
    cw2T = np.tile(np.ascontiguousarray(cw2r.T).astype(np.float16), (2, 1))

    scal = np.zeros((128, 14), np.float32)
    scal[:, 0] = a1[:128]; scal[:, 1] = a1[128:]
    scal[:, 2] = b1f[:128]; scal[:, 3] = b1f[128:]
    scal[:64, 4] = b1p; scal[:64, 5] = b2p
    scal[:64, 6] = b2f; scal[64:, 6] = b2f
    scal[:, 7] = cb2r
    scal[:, 8] = a3p[:128]; scal[:, 9] = a3p[128:]
    scal[:, 10] = b3fp[:128]; scal[:, 11] = b3fp[128:]
    scal[:, 12] = f["bc"][:128]; scal[:, 13] = f["bc"][128:]

    shared = dict(w1T=w1T.astype(bf16_np), w2T=w2T.astype(bf16_np),
                  w3T=w3T.astype(bf16_np), wcT=wcT.astype(bf16_np),
                  cw1T=cw1T, cw2T=cw2T, scal=scal)
    in_maps = []
    for core in range(8):
        b, i = divmod(core, 4)
        r0 = RB * i
        m = dict(shared)
        m["xp"] = np.ascontiguousarray(
            xpad[b].reshape(2, 128, H + 2 * PAD, WP)[:, :, r0:r0 + ROWS, :]
        ).astype(bf16_np)
        m["rsubp"] = np.ascontiguousarray(
            rsubp[:, :, r0:r0 + RB, :].reshape(2, K2, NQ))
        in_maps.append(m)
    return in_maps


def kernel(**inputs):
    from concourse.bass_utils import run_bass_kernel_spmd
    nc = _build_program()
    in_maps = _host_prep(inputs)
    res = run_bass_kernel_spmd(nc, in_maps, core_ids=list(range(8)))
    global LAST_RESULTS
    LAST_RESULTS = res
    y = np.zeros((B, C, H, W), np.float32)
    for core in range(8):
        b, i = divmod(core, 4)
        yc = res.results[core]["y"]
        y[b, :, RB * i:RB * (i + 1), :] = yc.reshape(C, RB, W)
    return y


# revision 18
# speedup vs baseline: 1.0415x; 1.0415x over previous
"""SAN Bottleneck (pairwise self-attention) Trainium2 kernel.

Sharding: 8 cores = 2 batches x 4 row-blocks of 14 rows (H=56). Each core
receives a reflect-padded input slice (20 rows x 62 cols), so the 7x7
unfold needs no runtime halo exchange and no edge special-casing.

Per-core pipeline (all batchnorms folded into per-channel scale/bias on host):
  bn1+relu -> x1/x2/x3 1x1 convs (matmuls)
  feat = relu(x1 - shifted-window(x2))            (fp16, DVE/GPSIMD)
  mm1 66->64 (fp16), PSUM packed 2 column-halves onto 128 partitions so
  the relu+bias eviction (ACT) covers half the free size
  mm2 64->128 with 4x-replicated head weights -> exp+bias (ACT)
  softmax normalizer + aggregation: shifted-window products and pairwise
  tree adds over the 49 taps, with 4-free-dim APs so each 7x{4,3}-tap
  group is a single DVE/GPSIMD instruction
  bn2+relu -> wc conv + bias + identity residual.

The x3/aggregation channels are permuted host-side (s-split: tile t,
partition p <-> channel 8*(p//4)+4t+(p%4)) so one 4x-replicated exp tensor
serves both 128-channel tiles without any partition broadcast. The position
branch (batch independent) is precomputed on host as relu(bn(subp)) and DMA'd
into feat rows 64:65. Input x ships as bf16 (residual tolerance allows it).
"""

import numpy as np
import ml_dtypes

bf16_np = ml_dtypes.bfloat16

K = 7
PAD = 3
EPS = 1e-5
B, C, H, W = 2, 256, 56, 56
RB = 14              # rows per core
NQ = RB * W          # 784
ROWS = RB + 2 * PAD  # 20
WP = W + 2 * PAD     # 62
K2 = K * K
CHUNKS = [(0, 4), (4, 4), (8, 3), (11, 3)]

_BUILD_CACHE = {}


def _perm_channels():
    perm = np.zeros(256, np.int64)
    for t in range(2):
        for p in range(128):
            perm[t * 128 + p] = 8 * (p // 4) + 4 * t + (p % 4)
    return perm


def _build_program():
    if "nc" in _BUILD_CACHE:
        return _BUILD_CACHE["nc"]
    import concourse.bass as bass
    import concourse.bacc as bacc
    import concourse.tile as tile
    import concourse.mybir as mybir
    from contextlib import ExitStack

    f32 = mybir.dt.float32
    f16 = mybir.dt.float16
    bf16 = mybir.dt.bfloat16
    Alu = mybir.AluOpType
    Act = mybir.ActivationFunctionType

    nc = bacc.Bacc("TRN2", target_bir_lowering=False, num_devices=8)

    xp_d = nc.dram_tensor("xp", [2, 128, ROWS, WP], bf16, kind="ExternalInput")
    rsubp_d = nc.dram_tensor("rsubp", [2, K2, NQ], f16, kind="ExternalInput")
    w1T_d = nc.dram_tensor("w1T", [2, 128, 64], bf16, kind="ExternalInput")
    w2T_d = nc.dram_tensor("w2T", [2, 128, 64], bf16, kind="ExternalInput")
    w3T_d = nc.dram_tensor("w3T", [2, 128, 2, 128], bf16, kind="ExternalInput")
    wcT_d = nc.dram_tensor("wcT", [2, 128, 2, 128], bf16, kind="ExternalInput")
    cw1T_d = nc.dram_tensor("cw1T", [66, 64], f16, kind="ExternalInput")
    cw2T_d = nc.dram_tensor("cw2T", [128, 128], f16, kind="ExternalInput")
    scal_d = nc.dram_tensor("scal", [128, 14], f32, kind="ExternalInput")
    y_d = nc.dram_tensor("y", [2, 128, RB, W], f32, kind="ExternalOutput")

    def win_ap(base, elem_off, ndj, nr):
        # [P][ndj dj (stride 2)][nr rows (stride WP)][56 cols] into a flat
        # padded [P, ROWS*WP] tile
        return bass.AP(
            tensor=base.tensor,
            offset=base.offset + elem_off,
            ap=[base.ap[0], [2, ndj], [WP, nr], [1, W]],
        )

    def kq_ap(base3, k0, ndj, nqc, nr):
        # [P][ndj (stride 2*nqc)][nr][56] into a [P, 49, nqc] tile at tap k0
        return bass.AP(
            tensor=base3.tensor,
            offset=base3.offset + k0 * nqc,
            ap=[base3.ap[0], [2 * nqc, ndj], [W, nr], [1, W]],
        )

    with tile.TileContext(nc) as tc, ExitStack() as stack:
        consts = stack.enter_context(tc.tile_pool(name="consts", bufs=1))
        xpp = stack.enter_context(tc.tile_pool(name="xpp", bufs=1))
        headsb = stack.enter_context(tc.tile_pool(name="headsb", bufs=1))

        w1s = consts.tile([128, 2, 64], bf16, tag="w1s")
        w2s = consts.tile([128, 2, 64], bf16, tag="w2s")
        w3s = consts.tile([128, 2, 2, 128], bf16, tag="w3s")
        wcs = consts.tile([128, 2, 2, 128], bf16, tag="wcs")
        cw1s = consts.tile([66, 64], f16, tag="cw1s")
        cw2s = consts.tile([128, 128], f16, tag="cw2s")
        scals = consts.tile([128, 14], f32, tag="scals")
        for kt in range(2):
            nc.sync.dma_start(out=w1s[:, kt, :], in_=w1T_d[kt])
            nc.sync.dma_start(out=w2s[:, kt, :], in_=w2T_d[kt])
            nc.sync.dma_start(out=w3s[:, kt, :, :], in_=w3T_d[kt])
            nc.sync.dma_start(out=wcs[:, kt, :, :], in_=wcT_d[kt])
        nc.sync.dma_start(out=cw1s[:], in_=cw1T_d[:])
        nc.sync.dma_start(out=cw2s[:], in_=cw2T_d[:])
        nc.sync.dma_start(out=scals[:], in_=scal_d[:])

        a1 = [scals[:, 0:1], scals[:, 1:2]]
        b1f = [scals[:, 2:3], scals[:, 3:4]]
        b1p = scals[0:64, 4:5]
        b2p = scals[0:64, 5:6]
        b2f = scals[:, 6:7]          # replicated to 128 rows host-side
        cb2r = scals[:, 7:8]
        a3p = [scals[:, 8:9], scals[:, 9:10]]
        b3fp = [scals[:, 10:11], scals[:, 11:12]]
        bcb = [scals[:, 12:13], scals[:, 13:14]]

        xps = [xpp.tile([128, ROWS, WP], bf16, tag=f"xp{t}", name=f"xp{t}") for t in range(2)]
        for t in range(2):
            nc.sync.dma_start(out=xps[t][:], in_=xp_d[t])
        obn = [headsb.tile([128, ROWS * WP], bf16, tag=f"obn{t}", name=f"obn{t}") for t in range(2)]
        for t in range(2):
            nc.scalar.activation(
                out=obn[t][:],
                in_=xps[t][:].rearrange("p r w -> p (r w)"),
                func=Act.Relu, bias=b1f[t], scale=a1[t])

        x1s = headsb.tile([64, RB, W], f16, tag="x1s")
        x2p = headsb.tile([64, ROWS * WP], f16, tag="x2p")
        x3ps = headsb.tile([128, 2, ROWS * WP], f16, tag="x3ps")

        with tc.tile_pool(name="pshead", bufs=4, space="PSUM") as pshead:
            ccuts = [(0, 416), (416, 416), (832, 408)]
            for (o0, n) in ccuts:
                ps = pshead.tile([64, 416], f32, tag="ps64")
                for kt in range(2):
                    nc.tensor.matmul(
                        ps[:, :n], w2s[:, kt, :],
                        obn[kt][:, o0:o0 + n],
                        start=(kt == 0), stop=(kt == 1))
                nc.scalar.activation(out=x2p[:, o0:o0 + n], in_=ps[:, :n],
                                     func=Act.Identity, bias=b2p, scale=1.0)
            for half in range(2):
                ps = pshead.tile([64, 416], f32, tag="ps64")
                for kt in range(2):
                    rhs = obn[kt][:].rearrange("p (r w) -> p r w", w=WP)[
                        :, 3 + 7 * half:3 + 7 * (half + 1), 3:3 + W]
                    nc.tensor.matmul(ps[:, :392], w1s[:, kt, :],
                                     rhs,
                                     start=(kt == 0), stop=(kt == 1))
                nc.scalar.activation(
                    out=x1s[:, 7 * half:7 * (half + 1), :],
                    in_=ps[:, :392].rearrange("p (r w) -> p r w", w=W),
                    func=Act.Identity, bias=b1p, scale=1.0)
            for ot in range(2):
                for (o0, n) in ccuts:
                    ps = pshead.tile([128, 416], f32, tag="ps128")
                    for kt in range(2):
                        nc.tensor.matmul(
                            ps[:, :n], w3s[:, kt, ot, :],
                            obn[kt][:, o0:o0 + n],
                            start=(kt == 0), stop=(kt == 1))
                    nc.scalar.activation(out=x3ps[:, ot, o0:o0 + n],
                                         in_=ps[:, :n], func=Act.Copy)

        featp = stack.enter_context(tc.tile_pool(name="featp", bufs=2))
        h2p = stack.enter_context(tc.tile_pool(name="h2p", bufs=1))
        e4p = stack.enter_context(tc.tile_pool(name="e4p", bufs=2))
        prodp = stack.enter_context(tc.tile_pool(name="prodp", bufs=1))
        smallp = stack.enter_context(tc.tile_pool(name="smallp", bufs=2))
        zscp = stack.enter_context(tc.tile_pool(name="zscp", bufs=1))
        ps1p = stack.enter_context(tc.tile_pool(name="ps1p", bufs=2, space="PSUM"))
        ps2p = stack.enter_context(tc.tile_pool(name="ps2p", bufs=2, space="PSUM"))

        chunk_state = {}

        def ksum_tree(eng, t):
            for (a, b, n) in [(0, 24, 24), (0, 12, 12), (0, 6, 6), (0, 3, 3)]:
                eng.tensor_tensor(out=t[:, a:a + n, :], in0=t[:, a:a + n, :],
                                  in1=t[:, b:b + n, :], op=Alu.add)
            for b in (1, 2, 48):
                eng.tensor_tensor(out=t[:, 0, :], in0=t[:, 0, :],
                                  in1=t[:, b, :], op=Alu.add)

        def h2_block(nb2, tail_base, m):
            # map global 512-col block m of mm1's output onto (psum-half
            # partition start, h2 column start) under the pack-2 layout
            t, r = divmod(m, 4)
            if t < nb2:
                return 64 * (r // 2), t * 1024 + 512 * (r % 2)
            mt = m - 4 * nb2
            return 0, tail_base + 512 * mt

        def phase1(ci):
            (r0c, nr) = CHUNKS[ci]
            nqc = nr * W
            vc = K2 * nqc
            nb2 = vc // 2048
            tail_base = nb2 * 1024
            tail = vc - nb2 * 2048
            feat = featp.tile([66, K2, nqc], f16, tag="feat", name=f"feat{ci}")
            nc.sync.dma_start(out=feat[64:66, :, :],
                                in_=rsubp_d[:, :, r0c * W:r0c * W + nqc])

            fv = feat[0:64]
            x1v = x1s[:, r0c:r0c + nr, :]
            for di in range(K):
                for par in range(2):
                    ndj = 4 if par == 0 else 3
                    x2w = win_ap(x2p[:], (r0c + di) * WP + par, ndj, nr)
                    x1w = bass.AP(tensor=x1v.tensor, offset=x1v.offset,
                                  ap=[x1v.ap[0], [0, ndj], x1v.ap[1], x1v.ap[2]])
                    outw = kq_ap(fv, di * K + par, ndj, nqc, nr)
                    eng = nc.vector if par == 0 else nc.gpsimd
                    eng.tensor_tensor(out=outw, in0=x1w, in1=x2w,
                                      op=Alu.subtract)
            for bq in range(4):
                ks = (K2 * bq) // 4, (K2 * (bq + 1)) // 4
                nc.vector.tensor_scalar_max(
                    out=feat[0:64, ks[0]:ks[1], :].rearrange("p a b -> p (a b)"),
                    in0=feat[0:64, ks[0]:ks[1], :].rearrange("p a b -> p (a b)"),
                    scalar1=0.0)

            featf = feat[:].rearrange("p a b -> p (a b)")
            h2 = h2p.tile([64, vc], f16, tag="h2")
            for j0 in range(0, vc, 1024):
                n = min(1024, vc - j0)
                ps1 = ps1p.tile([64, 1024], f32, tag="ps1")
                for s in range(0, n, 512):
                    sn = min(512, n - s)
                    nc.tensor.matmul(ps1[:, s:s + sn], cw1s[:],
                                     featf[:, j0 + s:j0 + s + sn],
                                     start=True, stop=True)
                nc.scalar.activation(out=h2[:, j0:j0 + n], in_=ps1[:, :n],
                                     func=Act.Relu, bias=b2f[0:64], scale=1.0)

            e4 = e4p.tile([128, K2, nqc], f16, tag="e4")
            e4f = e4[:].rearrange("p a b -> p (a b)")
            for j0 in range(0, vc, 1024):
                n = min(1024, vc - j0)
                ps2 = ps2p.tile([128, 1024], f32, tag="ps2")
                for s in range(0, n, 512):
                    sn = min(512, n - s)
                    nc.tensor.matmul(ps2[:, s:s + sn], cw2s[0:64, :],
                                     h2[:, j0 + s:j0 + s + sn],
                                     start=True, stop=True)
                nc.scalar.activation(out=e4f[:, j0:j0 + n], in_=ps2[:, :n],
                                     func=Act.Exp, bias=cb2r, scale=1.0)

            chunk_state[ci] = (e4,)

        def phase2(ci):
            (r0c, nr) = CHUNKS[ci]
            nqc = nr * W
            (e4,) = chunk_state[ci]
            prods = []
            for ot in range(2):
                prodt = prodp.tile([128, K2, nqc], f16, tag=f"prod{ot}",
                                   name=f"prod{ot}")
                prods.append(prodt)
                for di in range(K):
                    for par in range(2):
                        ndj = 4 if par == 0 else 3
                        sv = x3ps[:, ot, :]
                        k0 = di * K + par
                        x3w = bass.AP(
                            tensor=sv.tensor,
                            offset=sv.offset + (r0c + di) * WP + par,
                            ap=[sv.ap[0], [2, ndj], [WP, nr], [1, W]])
                        e4w = kq_ap(e4[:], k0, ndj, nqc, nr)
                        outw = kq_ap(prods[ot][:], k0, ndj, nqc, nr)
                        nc.vector.tensor_tensor(out=outw, in0=e4w, in1=x3w,
                                                op=Alu.mult)
            ksum_tree(nc.vector, prods[0])
            ksum_tree(nc.vector, prods[1])
            zsc = zscp.tile([128, 24, nqc], f16, tag="zsc", name=f"zsc{ci}")
            nc.gpsimd.tensor_tensor(out=zsc[:, :, :], in0=e4[:, 0:24, :],
                                    in1=e4[:, 24:48, :], op=Alu.add)
            for (a, b, n) in [(0, 12, 12), (0, 6, 6), (0, 3, 3)]:
                nc.gpsimd.tensor_tensor(out=zsc[:, a:a + n, :],
                                        in0=zsc[:, a:a + n, :],
                                        in1=zsc[:, b:b + n, :], op=Alu.add)
            for bb in (1, 2):
                nc.gpsimd.tensor_tensor(out=zsc[:, 0, :], in0=zsc[:, 0, :],
                                        in1=zsc[:, bb, :], op=Alu.add)
            nc.gpsimd.tensor_tensor(out=zsc[:, 0, :], in0=zsc[:, 0, :],
                                    in1=e4[:, 48, :], op=Alu.add)

            zf = smallp.tile([128, nqc], f32, tag="zf")
            rz = smallp.tile([128, nqc], f32, tag="rz")
            nc.vector.tensor_copy(out=zf[:], in_=zsc[:, 0, :])
            nc.vector.reciprocal(out=rz[:], in_=zf[:])

            outb = []
            for ot in range(2):
                ob = smallp.tile([128, nqc], f32, tag=f"ob{ot}", name=f"ob{ot}")
                ob2 = smallp.tile([128, nqc], bf16, tag=f"ob2{ot}", name=f"ob2{ot}")
                outb.append(ob2)
                nc.vector.scalar_tensor_tensor(
                    out=ob[:], in0=prods[ot][:, 0, :], scalar=1.0, in1=rz[:],
                    op0=Alu.mult, op1=Alu.mult)
                nc.scalar.activation(out=ob2[:], in_=ob[:], func=Act.Relu,
                                     bias=b3fp[ot], scale=a3p[ot])

            for oo in range(2):
                psw = ps2p.tile([128, 1024], f32, tag="ps2")
                for kt in range(2):
                    nc.tensor.matmul(psw[:, :nqc], wcs[:, kt, oo, :],
                                     outb[kt][:],
                                     start=(kt == 0), stop=(kt == 1))
                ysb = smallp.tile([128, nqc], f32, tag=f"ysb{oo}", name=f"ysb{oo}")
                xi = xps[oo][:, PAD + r0c:PAD + r0c + nr, PAD:PAD + W]
                nc.vector.scalar_tensor_tensor(
                    out=ysb[:], in0=psw[:, :nqc], scalar=bcb[oo], in1=xi,
                    op0=Alu.add, op1=Alu.add)
                nc.sync.dma_start(
                    out=y_d[oo][:, r0c:r0c + nr, :],
                    in_=ysb[:].rearrange("p (r w) -> p r w", w=W))

        phase1(0)
        for ci in range(1, len(CHUNKS)):
            phase1(ci)
            phase2(ci - 1)
        phase2(len(CHUNKS) - 1)

    nc.compile()
    _BUILD_CACHE["nc"] = nc
    return nc


def _host_prep(inputs):
    f = {k: np.asarray(v, np.float32) for k, v in inputs.items()}

    def fold(n):
        a = f[n + "_g"] / np.sqrt(f[n + "_rv"] + EPS)
        return a, f[n + "_b"] - f[n + "_rm"] * a

    a1, b1f = fold("bn1")
    ac, bc1 = fold("cwbn1")
    a2, b2f = fold("cwbn2")
    a3, b3f = fold("bn2")

    W1p = ac[:64, None] * f["w1"]
    b1p = ac[:64] * f["b1"] + bc1[:64]
    W2p = ac[:64, None] * f["w2"]
    b2p = ac[:64] * f["b2"]
    cw1p = a2[:, None] * f["cw1"]

    perm = _perm_channels()
    w3p = f["w3"][perm]
    a3p = a3[perm]
    b3fp = b3f[perm]
    rep = np.arange(128) // 4
    cw2r = f["cw2"][rep]
    cb2r = f["cb2"][rep]

    locw = np.tile(np.linspace(-1.0, 1.0, W, dtype=np.float32)[None, :], (H, 1))
    loch = np.tile(np.linspace(-1.0, 1.0, H, dtype=np.float32)[:, None], (1, W))
    loc = np.stack([locw, loch], 0)
    p = np.einsum("chw,oc->ohw", loc, f["pw"]) + f["pb"][:, None, None]
    pp = np.pad(p, ((0, 0), (PAD, PAD), (PAD, PAD)), mode="reflect")
    pu = np.stack([pp[:, i:i + H, j:j + W] for i in range(K) for j in range(K)], 1)
    subp = p[:, None] - pu
    rsubp = np.maximum(ac[64:66, None, None, None] * subp
                       + bc1[64:66, None, None, None], 0).astype(np.float16)

    xpad = np.pad(f["x"], ((0, 0), (0, 0), (PAD, PAD), (PAD, PAD)), mode="reflect")

    w1T = np.ascontiguousarray(W1p.T).reshape(2, 128, 64).copy()
    w2T = np.ascontiguousarray(W2p.T).reshape(2, 128, 64).copy()
    w3T = np.empty((2, 128, 2, 128), np.float32)
    wcT = np.empty((2, 128, 2, 128), np.float32)
    wc_perm = f["wc"][:, perm]
    for kt in range(2):
        for ot in range(2):
            w3T[kt, :, ot, :] = w3p[ot * 128:(ot + 1) * 128,
                                    kt * 128:(kt + 1) * 128].T
            wcT[kt, :, ot, :] = wc_perm[ot * 128:(ot + 1) * 128,
                                        kt * 128:(kt + 1) * 128].T
    cw1T = np.ascontiguousarray(cw1p.T).astype(np.float16)
    cw2T = np.tile(np.ascontiguousarray(cw2r.T).astype(np.float16), (2, 1))

    scal = np.zeros((128, 14), np.float32)
    scal[:, 0] = a1[:128]; scal[:, 1] = a1[128:]
    scal[:, 2] = b1f[:128]; scal[:, 3] = b1f[128:]
    scal[:64, 4] = b1p; scal[:64, 5] = b2p
    scal[:64, 6] = b2f; scal[64:, 6] = b2f
    scal[:, 7] = cb2r
    scal[:, 8] = a3p[:128]; scal[:, 9] = a3p[128:]
    scal[:, 10] = b3fp[:128]; scal[:, 11] = b3fp[128:]
    scal[:, 12] = f["bc"][:128]; scal[:, 13] = f["bc"][128:]

    shared = dict(w1T=w1T.astype(bf16_np), w2T=w2T.astype(bf16_np),
                  w3T=w3T.astype(bf16_np), wcT=wcT.astype(bf16_np),
                  cw1T=cw1T, cw2T=cw2T, scal=scal)
    in_maps = []
    for core in range(8):
        b, i = divmod(core, 4)
        r0 = RB * i
        m = dict(shared)
        m["xp"] = np.ascontiguousarray(
            xpad[b].reshape(2, 128, H + 2 * PAD, WP)[:, :, r0:r0 + ROWS, :]
        ).astype(bf16_np)
        m["rsubp"] = np.ascontiguousarray(
            rsubp[:, :, r0:r0 + RB, :].reshape(2, K2, NQ))
        in_maps.append(m)
    return in_maps


def kernel(**inputs):
    from concourse.bass_utils import run_bass_kernel_spmd
    nc = _build_program()
    in_maps = _host_prep(inputs)
    res = run_bass_kernel_spmd(nc, in_maps, core_ids=list(range(8)))
    global LAST_RESULTS
    LAST_RESULTS = res
    y = np.zeros((B, C, H, W), np.float32)
    for core in range(8):
        b, i = divmod(core, 4)
        yc = res.results[core]["y"]
        y[b, :, RB * i:RB * (i + 1), :] = yc.reshape(C, RB, W)
    return y
